# revision 8
# baseline (speedup 1.0000x reference)
"""CropPool2D Trainium2 kernel.

out[b, c] = mean of img_feats[b, c, y1:y2, x1:x2] for bbox (x1, y1, x2, y2).

Strategy (data-parallel over batch, 8 NeuronCores, 8 samples each):
  - Samples are regrouped into 8 "slots" of 8 (one sample per core per
    slot), clustered by crop height so each slot's static window shape
    (slot-max ch x slot-max cw) is tight. Every core runs the same SPMD
    program; the sample->(core, slot) permutation is undone on host.
  - DMA cost on TRN2 is dominated by a ~17ns fixed cost per descriptor
    (contiguous run), so per (slot, sample) ONE DMA loads a contiguous
    bf16 span per channel: img[s, c, yo*W+xo : ... + (h-1)*W + w]. The
    crop window is then the strided view [h rows x first w cols] of
    that span (wrapped columns are loaded but never read).
  - One packed broadcast DMA ships all 8 window masks (inv_area inside
    the crop, 0 on the slot-window overage; windows are clamped inside
    the image so overage is always finite image data - no pre-zeroing).
  - Per (slot, group-of-128-channels): one fused DVE
    scalar_tensor_tensor (window * mask) with accum_out -> the [128,1]
    per-channel crop mean. One final DMA scatters [128, 32] -> [8, 512].
"""

import numpy as np
import ml_dtypes

B, C, H, W = 64, 512, 56, 56
N_CORES = 8
BL = B // N_CORES  # samples per core == slots
P = 128
G = C // P  # channel groups per sample

_prog_cache: dict = {}

# Band DMAs alternate between the SP and ACT HWDGE rings; ring r owns
# slots r, r+2, r+4, r+6 and loads its 4 window offsets in one go.
_RING_SLOTS = [[0, 2, 4, 6], [1, 3, 5, 7]]


def _assign_slots(ch: np.ndarray, cw: np.ndarray):
    """Group the 64 samples into 8 slots of 8, one member per core.

    DMA cost is span-bound (~ slot-max ch), so sorting by crop height
    and taking octiles minimizes sum(max_ch). The secondary cw sort
    tightens max_cw for the DVE/mask cost.
    """
    order = np.lexsort((cw, ch))[::-1]  # ch desc, then cw desc
    groups = [order[j * N_CORES : (j + 1) * N_CORES] for j in range(BL)]
    shapes = [(int(ch[g].max()), int(cw[g].max())) for g in groups]
    return groups, shapes


def _build_program(shapes, unroll: int = 1):
    """Build + compile the SPMD Bass program for the 8 slot shapes."""
    import concourse.bacc as bacc
    import concourse.mybir as mybir
    import concourse.tile as tile
    from concourse.bass import ds

    f32 = mybir.dt.float32
    bf16 = mybir.dt.bfloat16
    i32 = mybir.dt.int32

    nc = bacc.Bacc("TRN2", target_bir_lowering=False, debug=False)

    areas = [h * w for h, w in shapes]
    spans = [(h - 1) * W + w for h, w in shapes]
    tot_area = sum(areas)
    moffs = np.cumsum([0] + areas).tolist()

    img = nc.dram_tensor("img", [BL, C, H, W], bf16, kind="ExternalInput").ap()
    # meta: flat window-start offsets (yo*W+xo), SP-ring slots then ACT's.
    meta = nc.dram_tensor("meta", [1, BL], i32, kind="ExternalInput").ap()
    # 0/1 window masks, host-side pre-broadcast across partitions so the
    # load streams at full rate (a to_broadcast DMA re-reads the source
    # per partition and crawls).
    maskd = nc.dram_tensor("mask", [P, tot_area], bf16, kind="ExternalInput").ap()
    # Per-slot 1/area, host-broadcast to all partitions.
    invd = nc.dram_tensor("inv", [P, BL], f32, kind="ExternalInput").ap()
    outd = nc.dram_tensor("out", [BL, C], f32, kind="ExternalOutput").ap()

    img_flat = img.rearrange("b c h w -> b c (h w)")

    with tile.TileContext(nc) as tc:
        with (
            tc.tile_pool(name="const", bufs=1) as constp,
            tc.tile_pool(name="bandp", bufs=1) as bandp,
            tc.tile_pool(name="prodp", bufs=2) as prodp,
            tc.tile_pool(name="outp", bufs=1) as outp,
        ):
            meta_sb = constp.tile([1, BL], i32)
            nc.sync.dma_start(meta_sb, meta)

            mask_sb = constp.tile([P, tot_area], bf16)
            nc.scalar.dma_start(mask_sb, maskd)

            inv_sb = constp.tile([P, BL], f32)
            nc.scalar.dma_start(inv_sb, invd)

            out_sb = outp.tile([P, BL * G], f32)

            rings = [
                (nc.sync, (mybir.EngineType.SP,)),
                (nc.scalar, (mybir.EngineType.Activation,)),
            ]
            # Band tiles hold full-width rows; the DMA writes the leading
            # span only, and compute reads only the first w columns of
            # each row, so the unwritten tail is never consumed.
            bands = [
                bandp.tile([P, G, h, W], bf16, name=f"band{j}", tag=f"band{j}")
                for j, (h, w) in enumerate(shapes)
            ]

            for _rep in range(unroll):
                offs = [None] * BL
                for r, (eng, etypes) in enumerate(rings):
                    _, vals = nc.values_load_multi_w_load_instructions(
                        meta_sb[0:1, 4 * r : 4 * (r + 1)],
                        engines=etypes,
                        min_val=0,
                        max_val=H * W - 1,
                        skip_runtime_bounds_check=True,
                    )
                    for k, j in enumerate(_RING_SLOTS[r]):
                        # Tighten the per-slot bound for the AP check.
                        offs[j] = nc.s_assert_within(
                            vals[k],
                            min_val=0,
                            max_val=H * W - spans[j],
                            skip_runtime_assert=True,
                        )

                for j, (h, w) in enumerate(shapes):
                    eng, _ = rings[j % 2]
                    dst = bands[j][:].rearrange("p g h x -> p (g h x)")
                    dst = dst.rearrange("p (g s) -> p g s", g=G)[:, :, 0 : spans[j]]
                    eng.dma_start(dst, img_flat[j, :, ds(offs[j], spans[j])])

                for j, (h, w) in enumerate(shapes):
                    mask_v = mask_sb[:, moffs[j] : moffs[j + 1]].rearrange(
                        "p (h w) -> p h w", w=w
                    )
                    for g in range(G):
                        prod = prodp.tile([P, h * w], bf16, name=f"prod{j}_{g}")
                        col = j * G + g
                        # out = (window * 1/area) * mask01; accum = sum(out).
                        nc.vector.scalar_tensor_tensor(
                            out=prod[:].rearrange("p (h w) -> p h w", w=w),
                            in0=bands[j][:, g, :, 0:w],
                            scalar=inv_sb[:, j : j + 1],
                            in1=mask_v,
                            op0=mybir.AluOpType.mult,
                            op1=mybir.AluOpType.mult,
                            accum_out=out_sb[:, col : col + 1],
                        )

            # out_sb[p, j*G+g] holds channel c = 4p+g of slot-sample j.
            nc.sync.dma_start(
                outd.rearrange("s (p g) -> p s g", g=G),
                out_sb[:].rearrange("p (s g) -> p s g", g=G),
            )

    nc.compile()
    return nc


def _host_prep(bboxes: np.ndarray):
    bb = np.asarray(bboxes).astype(np.int64)
    x1, y1, x2, y2 = bb[:, 0], bb[:, 1], bb[:, 2], bb[:, 3]
    ch = y2 - y1
    cw = x2 - x1
    assert (ch > 0).all() and (cw > 0).all(), "invalid bboxes"
    groups, shapes = _assign_slots(ch, cw)

    inv_area = 1.0 / (ch * cw).astype(np.float64)

    areas = [h * w for h, w in shapes]
    tot_area = sum(areas)
    meta = np.zeros((N_CORES, BL), np.int32)
    masks = np.zeros((N_CORES, tot_area), np.float32)
    invs = np.zeros((N_CORES, BL), np.float32)
    meta_pos = {j: r * 4 + k for r, sl in enumerate(_RING_SLOTS) for k, j in enumerate(sl)}
    off = 0
    for j, (hj, wj) in enumerate(shapes):
        g = groups[j]  # 8 sample ids, one per core
        ys = np.minimum(y1[g], H - hj)  # window start (clamped in-image)
        xs = np.minimum(x1[g], W - wj)
        meta[:, meta_pos[j]] = ys * W + xs
        invs[:, j] = inv_area[g]
        dy = (y1[g] - ys)[:, None, None]
        dx = (x1[g] - xs)[:, None, None]
        r = np.arange(hj)[None, :, None]
        c = np.arange(wj)[None, None, :]
        valid = (
            (r >= dy)
            & (r < dy + ch[g][:, None, None])
            & (c >= dx)
            & (c < dx + cw[g][:, None, None])
        )
        masks[:, off : off + hj * wj] = valid.reshape(N_CORES, hj * wj)
        off += hj * wj

    return groups, shapes, meta, masks.astype(ml_dtypes.bfloat16), invs


def _run(img_feats: np.ndarray, bboxes: np.ndarray, **spmd_kwargs):
    from concourse.bass_utils import run_bass_kernel_spmd

    img = np.asarray(img_feats)
    assert img.shape == (B, C, H, W), img.shape
    img16 = np.ascontiguousarray(img.astype(ml_dtypes.bfloat16))
    groups, shapes, meta, masks, invs = _host_prep(bboxes)

    key = tuple(shapes)
    if key not in _prog_cache:
        _prog_cache[key] = _build_program(list(shapes))
    nc = _prog_cache[key]

    in_maps = []
    for i in range(N_CORES):
        sample_ids = [groups[j][i] for j in range(BL)]
        in_maps.append(
            {
                "img": img16[sample_ids],
                "meta": meta[i : i + 1],
                "mask": np.ascontiguousarray(
                    np.broadcast_to(masks[i], (P,) + masks[i].shape)
                ),
                "inv": np.ascontiguousarray(
                    np.broadcast_to(invs[i], (P,) + invs[i].shape)
                ),
            }
        )

    res = run_bass_kernel_spmd(
        nc, in_maps, core_ids=list(range(N_CORES)), **spmd_kwargs
    )
    out = np.empty((B, C), np.float32)
    for i in range(N_CORES):
        core_out = res.results[i]["out"]  # [BL, C] in slot order
        for j in range(BL):
            out[groups[j][i]] = core_out[j]
    return out, res


def kernel(img_feats: np.ndarray, bboxes: np.ndarray) -> np.ndarray:
    out, _ = _run(img_feats, bboxes)
    return out
